# revision 108
# baseline (speedup 1.0000x reference)
"""Trainium2 Bass kernel for nn_Attention_5093831213465.

Reference computation (per sample, x_b: [256, 4096]):
  q = Wq @ x_b; k = maxpool2(Wk @ x_b); v = maxpool2(Wv @ x_b)
  attn = softmax_over_k(k^T @ q); y = gamma * Wa @ (v @ attn) + x_b
Sharding: data-parallel over batch, 2 samples per core on 8 cores.

Design notes (cost model: matmul = out-free-size x cycles/row, fp8
DoubleRow = 0.5/row; DVE/Act ~1 ns/free-elem + PSUM/SBUF access
overhead; every PSUM byte must be evacuated by Act or DVE, and
TensorTensor allows at most ONE PSUM operand):
- x, the q/k/v conv weights and y are bf16 (halves HBM traffic + SBUF;
  x f32->bf16 and y bf16->f32 conversions happen on host).  All DMA'd
  constants ride one uint8 blob transfer with bitcast views (each DMA
  costs ~650ns of serial queue-issue time); ones-valued tensors (kph
  shift row, on8/onb) are memset on the idle gpsimd engine.
- Attention logits are shifted via a 33rd contraction row (k row 32 =
  1, q row 32 = -(submax+2)); submax comes from a 64-key subsampled
  transposed attention, reduced in 2-chunk pieces.  exp -> fp8e5 on
  Act; odd chunks below 12 use a DVE Schraudolph exp for the last pair
  (bits16 = rne(184.665*z + 16250.5) bitcast bf16) to balance Act/DVE
  (the final chunks stay on Act because the drain is DVE-bound).
- Value matmul in fp8 DoubleRow (E e5m2 x vT e4m3); softmax
  denominators via stationary-E matmuls (out free size 1, ~zero PE
  cost), then reciprocal -> PE transpose -> SBUF gather DMA -> gpsimd
  partition_broadcast -> one DVE normalize per chunk (e4m3, pre-Wa).
- 2x2 maxpools are single fused tensor_reduces (the two-PSUM-operand
  rule blocks any cheaper TT-based split).
- Software pipeline with LAG=4: chunk tails (den chain, U, Wa,
  residual) are emitted four chunks behind their attention pairs so
  the in-order PE never waits on the exp stream; the last 5 tails
  route pu/po to the (by then idle) pair pool.  Keeping the conv and
  attention phases separate is deliberate: conv tiles share the 3-slot
  pair PSUM ring, and any interleaving couples the pair matmuls (the
  Act exp feed) to DVE maxpool latencies.  More than ~7 Schraudolph
  units also jams that ring (the sch pair's PSUM slot is freed by the
  slow DVE instead of Act, starving the next job's pairs).
- Drain relief: the last 6 tails' second residual half is evacuated by
  an Act copy (Act idles in the drain) so its add runs in 2x bf16 mode
  on the DVE, overlapping the first half's full-rate add.
- Conv chunk-pairing: each conv tile holds TWO chunks (maxpool reduces
  and qs copies at free-size 1024); submax pieces cover 4 chunks.
- Drain hoist: the den/recip/broadcast chains AND the U/normalize of
  tails 12-15 are emitted at tail 11 (their exps all exist by then), so
  the drain overlaps the last heads and only runs Wa/residual/store.
  The last 5 tails' second residual half is evacuated by an Act copy
  (Act idles there) so its add runs in 2x bf16 mode on the DVE.
Timing: 105622 ns TimelineSim (session baseline 121053); rel err
1.016e-2 on hardware (gate 2e-2; y/x bf16 rounding adds ~1.6e-3).
"""

import sys

import numpy as np

if "/opt/trn_rl_repo" not in sys.path:
    sys.path.insert(0, "/opt/trn_rl_repo")

B, C, H, W = 16, 256, 64, 64
CA = C // 8          # 32  attn channels
CS = C // 2          # 128 value channels
HWF = H * W          # 4096 spatial positions
HWP = HWF // 4       # 1024 pooled positions
SPC = 2              # samples per core
NCORES = 8
CHUNK = 512
NCHUNK = HWF // CHUNK       # 8
KT = HWP // 128             # 8 kk tiles of 128
NPAIR = KT // 2             # 4 exp/U pairs per chunk
SHIFT_DELTA = 2.0           # c = submax + delta

_built = {}


def _build_program():
    from contextlib import ExitStack

    import concourse.bass as bass
    import concourse.tile as tile
    from concourse import bacc, mybir

    f32 = mybir.dt.float32
    bf16 = mybir.dt.bfloat16
    e4 = mybir.dt.float8e4
    e5 = mybir.dt.float8e5
    i16 = mybir.dt.int16
    u8 = mybir.dt.uint8
    DR = mybir.MatmulPerfMode.DoubleRow
    Exp = mybir.ActivationFunctionType.Exp
    Mult = mybir.AluOpType.mult
    Add = mybir.AluOpType.add
    Max = mybir.AluOpType.max

    nc = bacc.Bacc(
        "TRN2", target_bir_lowering=False, debug=False, enable_asserts=False
    )

    x_d = nc.dram_tensor("x", [SPC, 2, 128, HWF], bf16, kind="ExternalInput").ap()
    cb_d = nc.dram_tensor("cblob", [128, 2048], u8, kind="ExternalInput").ap()
    y_d = nc.dram_tensor("y", [SPC, 2, 128, HWF], bf16, kind="ExternalOutput").ap()

    with tile.TileContext(nc) as tc, ExitStack() as ctx:
        consts = ctx.enter_context(tc.tile_pool(name="consts", bufs=1))
        xp = ctx.enter_context(tc.tile_pool(name="xp", bufs=2))
        qsp = ctx.enter_context(tc.tile_pool(name="qsp", bufs=2))
        kvp = ctx.enter_context(tc.tile_pool(name="kvp", bufs=2))
        cm = ctx.enter_context(tc.tile_pool(name="cm", bufs=2))
        plp = ctx.enter_context(tc.tile_pool(name="plp", bufs=3))
        ep = ctx.enter_context(tc.tile_pool(name="ep", bufs=20))
        rp = ctx.enter_context(tc.tile_pool(name="rp", bufs=6))
        up = ctx.enter_context(tc.tile_pool(name="up", bufs=6))
        yp = ctx.enter_context(tc.tile_pool(name="yp", bufs=6))
        # PSUM budget (16KB/partition): pBig 3x[128,2,512]f32 (12KB) shared
        # by conv tiles and attn pairs, pWu 1x[128,512]f32 (2KB) pu/po ring,
        # pWsm small transposes/den.
        pBig = ctx.enter_context(tc.tile_pool(name="pBig", bufs=3, space="PSUM"))
        pWu = ctx.enter_context(tc.tile_pool(name="pWu", bufs=1, space="PSUM"))
        pWsm = ctx.enter_context(tc.tile_pool(name="pWsm", bufs=1, space="PSUM"))

        blob = consts.tile([128, 2048], u8)
        nc.sync.dma_start(blob[:], cb_d)
        wqk = blob[:, 0:256].bitcast(bf16).rearrange("p (t m) -> p t m", t=2)
        wv = blob[:, 256:768].bitcast(bf16).rearrange("p (t m) -> p t m", t=2)
        wa = blob[:, 768:1280].bitcast(bf16).rearrange("p (t m) -> p t m", t=2)
        idb = blob[:, 1280:1536].bitcast(bf16)
        idf = blob[:, 1536:2048].bitcast(f32)
        on8 = consts.tile([128, 2, 1], e5)
        nc.gpsimd.memset(on8[:], 1.0)
        onb = consts.tile([128, 1], bf16)
        nc.gpsimd.memset(onb[:], 1.0)

        xrs = []
        for s in range(SPC):
            xr = xp.tile([128, 2, HWF], bf16, tag="xr", name=f"xr{s}")
            xrs.append(xr)

        def load_x(s, slices):
            for lo, hi in slices:
                for t in range(2):
                    nc.sync.dma_start(
                        xrs[s][:, t, lo:hi], x_d[s, t, :, lo:hi]
                    )

        qs_l, kph_l, vT_l, vTb_l = [], [], [], []
        for s in range(SPC):
            qs_l.append(qsp.tile([33, KT, CHUNK], bf16, name=f"qs{s}", tag="qs"))
            kph_l.append(kvp.tile([33, KT, 128], bf16, name=f"kph{s}", tag="kph"))
            vT_l.append(kvp.tile([128, NPAIR, 2, 128], e4, name=f"vT{s}", tag="vT"))
            vTb_l.append(kvp.tile([128, 2, 128], bf16, name=f"vTb{s}", tag="vTb"))
        vph_l = [kvp.tile([128, KT, 128], bf16, name=f"vph{s}", tag="vph") for s in range(SPC)]

        # constant-one shift row of kph, set on the idle gpsimd engine
        for s in range(SPC):
            nc.gpsimd.memset(kph_l[s][32:33, :, :], 1.0)

        def conv_chunk2(s, ck):
            # two chunks per conv tile: the maxpool reduces and the qs copy
            # run at free-size 1024, amortizing PSUM/SBUF access overheads
            qs, kph, vph = qs_l[s], kph_l[s], vph_l[s]
            vT, vTb = vT_l[s], vTb_l[s]
            pq2 = pBig.tile([128, 2, CHUNK], f32, tag="big", name=f"pq{s}_{ck}")
            for h in range(2):
                cs = slice((ck + h) * CHUNK, (ck + h + 1) * CHUNK)
                for t in range(2):
                    nc.tensor.matmul(
                        pq2[0:64, h, :], wqk[:, t, :], xrs[s][:, t, cs],
                        start=(t == 0), stop=(t == 1),
                    )
            nc.scalar.copy(qs[0:32, ck : ck + 2, :], pq2[0:32, :, :])
            nc.vector.tensor_reduce(
                kph[0:32, ck : ck + 2, :].rearrange(
                    "p c (h2 w2) -> p c h2 w2", h2=4
                ),
                pq2[32:64, :, :].rearrange(
                    "p c (h2 dh w2 dw) -> p c h2 w2 dh dw",
                    h2=4, dh=2, w2=32, dw=2,
                ),
                axis=mybir.AxisListType.XY, op=Max,
            )
            pv2 = pBig.tile([128, 2, CHUNK], f32, tag="big", name=f"pv{s}_{ck}")
            for h in range(2):
                cs = slice((ck + h) * CHUNK, (ck + h + 1) * CHUNK)
                for t in range(2):
                    nc.tensor.matmul(
                        pv2[:, h, :], wv[:, t, :], xrs[s][:, t, cs],
                        start=(t == 0), stop=(t == 1),
                    )
            nc.vector.tensor_reduce(
                vph[:, ck : ck + 2, :].rearrange(
                    "p c (h2 w2) -> p c h2 w2", h2=4
                ),
                pv2[:, :, :].rearrange(
                    "p c (h2 dh w2 dw) -> p c h2 w2 dh dw",
                    h2=4, dh=2, w2=32, dw=2,
                ),
                axis=mybir.AxisListType.XY, op=Max,
            )
            for h in range(2):
                ptr = pWsm.tile([128, 128], bf16, tag="sm")
                nc.tensor.transpose(ptr[:], vph[:, ck + h, :], idb[:])
                nc.scalar.copy(vT[:, (ck + h) // 2, (ck + h) % 2, :], ptr[:])
                if ck + h >= KT - 2:
                    nc.scalar.copy(vTb[:, (ck + h) % 2, :], ptr[:])

        def submax_phase(s):
            qs, kph = qs_l[s], kph_l[s]
            ksub = kph[0:32, :, :].rearrange(
                "p kt (j v) -> p kt j v", v=16
            )[:, :, :, 0]
            cmax = cm.tile([128, 32], bf16, tag="cmax")
            for ck in range(0, NCHUNK, 4):
                psm = pBig.tile([128, 16, 64], f32, tag="big")
                for jj in range(16):
                    c2, j = ck + jj // 4, jj % 4
                    nc.tensor.matmul(
                        psm[:, jj, :],
                        qs[0:32, c2, j * 128 : (j + 1) * 128],
                        ksub,
                        start=True, stop=True,
                    )
                nc.vector.tensor_reduce(
                    cmax[:, ck * 4 : ck * 4 + 16],
                    psm[:],
                    axis=mybir.AxisListType.X, op=Max,
                )
            cneg = cm.tile([128, 32], bf16, tag="cneg")
            nc.vector.tensor_scalar(
                cneg[:], cmax[:], -1.0, -SHIFT_DELTA, Mult, Add
            )
            pcn = pWsm.tile([32, 128], bf16, tag="sm")
            nc.tensor.transpose(pcn[:], cneg[:], idb[:])
            cnT = cm.tile([32, 128], bf16, tag="cnT")
            nc.vector.tensor_copy(cnT[:], pcn[:])
            nc.sync.dma_start(
                qs[32:33, :, :].rearrange("o kt (j m) -> o (kt j) m", j=4),
                cnT[:],
            )

        # ---- attention phases: software pipeline, tails LAG chunks behind
        # their heads so the in-order PE never blocks on the Act exp stream.
        jobs = [(s, ck) for s in range(SPC) for ck in range(NCHUNK)]
        LAG = 4
        pend = {}

        def emit_head(i):
            s, ck = jobs[i]
            qs, kph = qs_l[s], kph_l[s]
            egs = []
            for g in range(NPAIR):
                pa = pBig.tile([128, 2, CHUNK], f32, tag="big")
                for t in range(2):
                    nc.tensor.matmul(
                        pa[:, t, :],
                        kph[:, 2 * g + t, :],
                        qs[:, ck, :],
                        start=True, stop=True,
                    )
                if g < NPAIR - 1 or (i % 2 == 0) or i >= 14:
                    eg = ep.tile([128, 2, CHUNK], e5, tag="E")
                    nc.scalar.activation(eg[:], pa[:], Exp)
                else:
                    # Schraudolph exp in bf16 bits on the DVE:
                    # bits16 = rne(z*184.665 + 16250.5); bitcast -> bf16
                    eg = ep.tile([128, 2, CHUNK], i16, tag="E")
                    nc.vector.tensor_scalar(
                        eg[:], pa[:], 184.6650, 16250.5, Mult, Add
                    )
                    eg = eg.bitcast(bf16)
                egs.append(eg)
            pend[i] = (egs, i % 2 != 0 and i < 14)

        rbs = {}

        def emit_chain(i):
            egs, last_bf = pend[i]
            den = pWsm.tile([128, 4], f32, tag="sm", name=f"den{i}")
            for j in range(4):
                for g in range(NPAIR - 1):
                    nc.tensor.matmul(
                        den[:, j : j + 1],
                        egs[g][:, :, j * 128 : (j + 1) * 128],
                        on8[:],
                        start=(g == 0), stop=False,
                        perf_mode=DR,
                    )
                if last_bf:
                    for t in range(2):
                        nc.tensor.matmul(
                            den[:, j : j + 1],
                            egs[NPAIR - 1][:, t, j * 128 : (j + 1) * 128],
                            onb[:],
                            start=False, stop=(t == 1),
                        )
                else:
                    nc.tensor.matmul(
                        den[:, j : j + 1],
                        egs[NPAIR - 1][:, :, j * 128 : (j + 1) * 128],
                        on8[:],
                        start=False, stop=True,
                        perf_mode=DR,
                    )
            r4 = rp.tile([128, 4], f32, tag="r4", name=f"r4_{i}")
            nc.vector.reciprocal_approx_fast(r4[:], den[:])
            prT = pWsm.tile([4, 128], f32, tag="sm", name=f"prT{i}")
            nc.tensor.transpose(prT[:], r4[:], idf[:])
            rr4 = rp.tile([4, 128], f32, tag="rr4", name=f"rr4_{i}")
            nc.vector.tensor_copy(rr4[:], prT[:])
            rrow = rp.tile([1, CHUNK], f32, tag="rrow", name=f"rrow{i}")
            nc.sync.dma_start(
                rrow[0:1, :].rearrange("o (j m) -> o j m", j=4), rr4[:]
            )
            rb = rp.tile([128, CHUNK], f32, tag="rb", name=f"rb{i}")
            nc.gpsimd.partition_broadcast(rb[:], rrow[0:1, :])
            rbs[i] = rb

        uns = {}

        def emit_U(i):
            s, ck = jobs[i]
            egs, last_bf = pend[i]
            vT = vT_l[s]
            rb = rbs.pop(i)
            pT = pBig if i >= len(jobs) - 5 else pWu
            pu = pT.tile([128, CHUNK], f32, name=f"pu{i}", tag="u" if pT is pWu else "big")
            for g in range(NPAIR - 1):
                nc.tensor.matmul(
                    pu[:], vT[:, g, :, :], egs[g][:],
                    start=(g == 0), stop=False,
                    perf_mode=DR,
                )
            if last_bf:
                vTb = vTb_l[s]
                for t in range(2):
                    nc.tensor.matmul(
                        pu[:], vTb[:, t, :], egs[NPAIR - 1][:, t, :],
                        start=False, stop=(t == 1),
                    )
            else:
                nc.tensor.matmul(
                    pu[:], vT[:, NPAIR - 1, :, :], egs[NPAIR - 1][:],
                    start=False, stop=True,
                    perf_mode=DR,
                )
            un = up.tile([128, CHUNK], e4, tag="un", name=f"un{i}")
            nc.vector.tensor_mul(un[:], pu[:], rb[:])
            uns[i] = un

        def emit_tail(i):
            s, ck = jobs[i]
            if i not in rbs and i not in uns:
                emit_chain(i)
            cs = slice(ck * CHUNK, (ck + 1) * CHUNK)
            if i not in uns:
                emit_U(i)
            un = uns.pop(i)
            pend.pop(i)
            if i == 11:
                # drain hoist: chains AND U/normalize of tails 12-15 emitted
                # here so they overlap the last heads' exps; the drain then
                # only runs Wa/residual/store per tail
                for j2 in range(12, len(jobs)):
                    emit_chain(j2)
                for j2 in range(12, len(jobs)):
                    emit_U(j2)
            pT = pBig if i >= len(jobs) - 5 else pWu
            yt = yp.tile([128, 2, CHUNK], bf16, tag="y")
            drain = i >= len(jobs) - 5
            for mt in range(2):
                po = pT.tile([128, CHUNK], f32, name=f"po{i}_{mt}", tag="u" if pT is pWu else "big")
                nc.tensor.matmul(
                    po[:], wa[:, mt, :], un[:],
                    start=True, stop=True,
                )
                if drain and mt == 1:
                    # drain is DVE-bound while Act idles: evacuate the second
                    # po on Act (in parallel with DVE's first residual add),
                    # then its residual add runs in 2x bf16 mode on DVE
                    tp = yp.tile(
                        [128, CHUNK], bf16, tag="ytmp", name=f"ytmp{i}_{mt}"
                    )
                    nc.scalar.copy(tp[:], po[:])
                    nc.vector.tensor_add(
                        yt[:, mt, :], tp[:], xrs[s][:, mt, cs]
                    )
                else:
                    nc.vector.tensor_add(
                        yt[:, mt, :], po[:], xrs[s][:, mt, cs]
                    )
            nc.sync.dma_start(
                y_d[s, :, :, cs].rearrange("t p m -> p t m"), yt[:]
            )

        load_x(0, ((0, 1024), (1024, 2048), (2048, 4096)))
        for ck in range(0, NCHUNK, 2):
            conv_chunk2(0, ck)
        submax_phase(0)
        load_x(1, ((0, 1024), (1024, 2048), (2048, 4096)))
        for ck in range(0, NCHUNK, 2):
            conv_chunk2(1, ck)
        submax_phase(1)
        for i in range(len(jobs) + LAG):
            if i < len(jobs):
                emit_head(i)
            if i >= LAG:
                emit_tail(i - LAG)

    nc.compile()
    return nc


def _get_program():
    if "nc" not in _built:
        _built["nc"] = _build_program()
    return _built["nc"]


def _make_in_maps(x, Wq, Wk, Wv, Wa, gamma):
    import ml_dtypes

    x = np.ascontiguousarray(
        np.asarray(x, dtype=np.float32)
        .astype(ml_dtypes.bfloat16)
        .reshape(B, 2, 128, HWF)
    )
    wqkT = np.concatenate([np.asarray(Wq), np.asarray(Wk)], axis=0).T
    wqkT = np.ascontiguousarray(
        wqkT.reshape(2, 128, 64).transpose(1, 0, 2).astype(ml_dtypes.bfloat16)
    )
    wvT = np.ascontiguousarray(
        np.asarray(Wv).T.reshape(2, 128, 128)
        .transpose(1, 0, 2).astype(ml_dtypes.bfloat16)
    )
    g = float(np.asarray(gamma).reshape(-1)[0])
    waT = np.ascontiguousarray(
        (g * np.asarray(Wa)).T.reshape(128, 2, 128).astype(ml_dtypes.bfloat16)
    )
    identB = np.eye(128, dtype=np.float32).astype(ml_dtypes.bfloat16)
    identF = np.eye(128, dtype=np.float32)
    blob = np.zeros((128, 2048), dtype=np.uint8)
    blob[:, 0:256] = wqkT.view(np.uint8).reshape(128, 256)
    blob[:, 256:768] = wvT.view(np.uint8).reshape(128, 512)
    blob[:, 768:1280] = waT.view(np.uint8).reshape(128, 512)
    blob[:, 1280:1536] = identB.view(np.uint8).reshape(128, 256)
    blob[:, 1536:2048] = identF.view(np.uint8).reshape(128, 512)
    return [
        {
            "x": np.ascontiguousarray(x[c * SPC : (c + 1) * SPC]),
            "cblob": blob,
        }
        for c in range(NCORES)
    ]


def kernel(x, Wq, Wk, Wv, Wa, gamma):
    from concourse import bass_utils

    nc = _get_program()
    in_maps = _make_in_maps(x, Wq, Wk, Wv, Wa, gamma)
    res = bass_utils.run_bass_kernel_spmd(
        nc, in_maps, core_ids=list(range(NCORES))
    )
    out = np.concatenate(
        [
            np.asarray(res.results[c]["y"]).astype(np.float32).reshape(1, SPC, C, HWF)
            for c in range(NCORES)
        ],
        axis=0,
    ).reshape(B, C, H, W)
    return out


# revision 109
# speedup vs baseline: 1.0049x; 1.0049x over previous
"""Trainium2 Bass kernel for nn_Attention_5093831213465.

Reference computation (per sample, x_b: [256, 4096]):
  q = Wq @ x_b; k = maxpool2(Wk @ x_b); v = maxpool2(Wv @ x_b)
  attn = softmax_over_k(k^T @ q); y = gamma * Wa @ (v @ attn) + x_b
Sharding: data-parallel over batch, 2 samples per core on 8 cores.

Design notes (cost model: matmul = out-free-size x cycles/row, fp8
DoubleRow = 0.5/row; DVE/Act ~1 ns/free-elem + PSUM/SBUF access
overhead; every PSUM byte must be evacuated by Act or DVE, and
TensorTensor allows at most ONE PSUM operand):
- x, the q/k/v conv weights and y are bf16 (halves HBM traffic + SBUF;
  x f32->bf16 and y bf16->f32 conversions happen on host).  All DMA'd
  constants ride one uint8 blob transfer with bitcast views (each DMA
  costs ~650ns of serial queue-issue time); ones-valued tensors (kph
  shift row, on8/onb) are memset on the idle gpsimd engine.
- Attention logits are shifted via a 33rd contraction row (k row 32 =
  1, q row 32 = -(submax+2)); submax comes from a 64-key subsampled
  transposed attention, reduced in 2-chunk pieces.  exp -> fp8e5 on
  Act; odd chunks below 12 use a DVE Schraudolph exp for the last pair
  (bits16 = rne(184.665*z + 16250.5) bitcast bf16) to balance Act/DVE
  (the final chunks stay on Act because the drain is DVE-bound).
- Value matmul in fp8 DoubleRow (E e5m2 x vT e4m3); softmax
  denominators via stationary-E matmuls (out free size 1, ~zero PE
  cost), then reciprocal -> PE transpose -> SBUF gather DMA -> gpsimd
  partition_broadcast -> one DVE normalize per chunk (e4m3, pre-Wa).
- 2x2 maxpools are single fused tensor_reduces (the two-PSUM-operand
  rule blocks any cheaper TT-based split).
- Software pipeline with LAG=4: chunk tails (den chain, U, Wa,
  residual) are emitted four chunks behind their attention pairs so
  the in-order PE never waits on the exp stream; the last 5 tails
  route pu/po to the (by then idle) pair pool.  Keeping the conv and
  attention phases separate is deliberate: conv tiles share the 3-slot
  pair PSUM ring, and any interleaving couples the pair matmuls (the
  Act exp feed) to DVE maxpool latencies.  More than ~7 Schraudolph
  units also jams that ring (the sch pair's PSUM slot is freed by the
  slow DVE instead of Act, starving the next job's pairs).
- Drain relief: the last 6 tails' second residual half is evacuated by
  an Act copy (Act idles in the drain) so its add runs in 2x bf16 mode
  on the DVE, overlapping the first half's full-rate add.
- Conv chunk-pairing: each conv tile holds TWO chunks (maxpool reduces
  and qs copies at free-size 1024); submax pieces cover 4 chunks.
- Drain hoist: the den/recip/broadcast chains AND the U/normalize of
  tails 12-15 are emitted at tail 11 (their exps all exist by then), so
  the drain overlaps the last heads and only runs Wa/residual/store.
  The last 5 tails' second residual half is evacuated by an Act copy
  (Act idles there) so its add runs in 2x bf16 mode on the DVE.
Timing: 105622 ns TimelineSim (session baseline 121053); rel err
1.016e-2 on hardware (gate 2e-2; y/x bf16 rounding adds ~1.6e-3).
"""

import sys

import numpy as np

if "/opt/trn_rl_repo" not in sys.path:
    sys.path.insert(0, "/opt/trn_rl_repo")

B, C, H, W = 16, 256, 64, 64
CA = C // 8          # 32  attn channels
CS = C // 2          # 128 value channels
HWF = H * W          # 4096 spatial positions
HWP = HWF // 4       # 1024 pooled positions
SPC = 2              # samples per core
NCORES = 8
CHUNK = 512
NCHUNK = HWF // CHUNK       # 8
KT = HWP // 128             # 8 kk tiles of 128
NPAIR = KT // 2             # 4 exp/U pairs per chunk
SHIFT_DELTA = 2.0           # c = submax + delta

_built = {}


def _build_program():
    from contextlib import ExitStack

    import concourse.bass as bass
    import concourse.tile as tile
    from concourse import bacc, mybir

    f32 = mybir.dt.float32
    bf16 = mybir.dt.bfloat16
    e4 = mybir.dt.float8e4
    e5 = mybir.dt.float8e5
    i16 = mybir.dt.int16
    u8 = mybir.dt.uint8
    DR = mybir.MatmulPerfMode.DoubleRow
    Exp = mybir.ActivationFunctionType.Exp
    Mult = mybir.AluOpType.mult
    Add = mybir.AluOpType.add
    Max = mybir.AluOpType.max

    nc = bacc.Bacc(
        "TRN2", target_bir_lowering=False, debug=False, enable_asserts=False
    )

    x_d = nc.dram_tensor("x", [SPC, 2, 128, HWF], bf16, kind="ExternalInput").ap()
    cb_d = nc.dram_tensor("cblob", [128, 2048], u8, kind="ExternalInput").ap()
    y_d = nc.dram_tensor("y", [SPC, 2, 128, HWF], bf16, kind="ExternalOutput").ap()

    with tile.TileContext(nc) as tc, ExitStack() as ctx:
        consts = ctx.enter_context(tc.tile_pool(name="consts", bufs=1))
        xp = ctx.enter_context(tc.tile_pool(name="xp", bufs=2))
        qsp = ctx.enter_context(tc.tile_pool(name="qsp", bufs=2))
        kvp = ctx.enter_context(tc.tile_pool(name="kvp", bufs=2))
        cm = ctx.enter_context(tc.tile_pool(name="cm", bufs=2))
        plp = ctx.enter_context(tc.tile_pool(name="plp", bufs=3))
        ep = ctx.enter_context(tc.tile_pool(name="ep", bufs=20))
        rp = ctx.enter_context(tc.tile_pool(name="rp", bufs=6))
        up = ctx.enter_context(tc.tile_pool(name="up", bufs=6))
        yp = ctx.enter_context(tc.tile_pool(name="yp", bufs=6))
        # PSUM budget (16KB/partition): pBig 3x[128,2,512]f32 (12KB) shared
        # by conv tiles and attn pairs, pWu 1x[128,512]f32 (2KB) pu/po ring,
        # pWsm small transposes/den.
        pBig = ctx.enter_context(tc.tile_pool(name="pBig", bufs=3, space="PSUM"))
        pWu = ctx.enter_context(tc.tile_pool(name="pWu", bufs=1, space="PSUM"))
        pWsm = ctx.enter_context(tc.tile_pool(name="pWsm", bufs=1, space="PSUM"))

        blob = consts.tile([128, 2048], u8)
        nc.sync.dma_start(blob[:], cb_d)
        wqk = blob[:, 0:256].bitcast(bf16).rearrange("p (t m) -> p t m", t=2)
        wv = blob[:, 256:768].bitcast(bf16).rearrange("p (t m) -> p t m", t=2)
        wa = blob[:, 768:1280].bitcast(bf16).rearrange("p (t m) -> p t m", t=2)
        idb = blob[:, 1280:1536].bitcast(bf16)
        idf = blob[:, 1536:2048].bitcast(f32)
        on8 = consts.tile([128, 2, 1], e5)
        nc.gpsimd.memset(on8[:], 1.0)
        onb = consts.tile([128, 1], bf16)
        nc.gpsimd.memset(onb[:], 1.0)

        xrs = []
        for s in range(SPC):
            xr = xp.tile([128, 2, HWF], bf16, tag="xr", name=f"xr{s}")
            xrs.append(xr)

        def load_x(s, slices):
            for lo, hi in slices:
                for t in range(2):
                    nc.sync.dma_start(
                        xrs[s][:, t, lo:hi], x_d[s, t, :, lo:hi]
                    )

        qs_l, kph_l, vT_l, vTb_l = [], [], [], []
        for s in range(SPC):
            qs_l.append(qsp.tile([33, KT, CHUNK], bf16, name=f"qs{s}", tag="qs"))
            kph_l.append(kvp.tile([33, KT, 128], bf16, name=f"kph{s}", tag="kph"))
            vT_l.append(kvp.tile([128, NPAIR, 2, 128], e4, name=f"vT{s}", tag="vT"))
            vTb_l.append(kvp.tile([128, 2, 128], bf16, name=f"vTb{s}", tag="vTb"))
        vph_l = [kvp.tile([128, KT, 128], bf16, name=f"vph{s}", tag="vph") for s in range(SPC)]

        # constant-one shift row of kph, set on the idle gpsimd engine
        for s in range(SPC):
            nc.gpsimd.memset(kph_l[s][32:33, :, :], 1.0)

        def conv_chunk2(s, ck):
            # two chunks per conv tile: the maxpool reduces and the qs copy
            # run at free-size 1024, amortizing PSUM/SBUF access overheads
            qs, kph, vph = qs_l[s], kph_l[s], vph_l[s]
            vT, vTb = vT_l[s], vTb_l[s]
            pq2 = pBig.tile([128, 2, CHUNK], f32, tag="big", name=f"pq{s}_{ck}")
            for h in range(2):
                cs = slice((ck + h) * CHUNK, (ck + h + 1) * CHUNK)
                for t in range(2):
                    nc.tensor.matmul(
                        pq2[0:64, h, :], wqk[:, t, :], xrs[s][:, t, cs],
                        start=(t == 0), stop=(t == 1),
                    )
            nc.scalar.copy(qs[0:32, ck : ck + 2, :], pq2[0:32, :, :])
            nc.vector.tensor_reduce(
                kph[0:32, ck : ck + 2, :].rearrange(
                    "p c (h2 w2) -> p c h2 w2", h2=4
                ),
                pq2[32:64, :, :].rearrange(
                    "p c (h2 dh w2 dw) -> p c h2 w2 dh dw",
                    h2=4, dh=2, w2=32, dw=2,
                ),
                axis=mybir.AxisListType.XY, op=Max,
            )
            pv2 = pBig.tile([128, 2, CHUNK], f32, tag="big", name=f"pv{s}_{ck}")
            for h in range(2):
                cs = slice((ck + h) * CHUNK, (ck + h + 1) * CHUNK)
                for t in range(2):
                    nc.tensor.matmul(
                        pv2[:, h, :], wv[:, t, :], xrs[s][:, t, cs],
                        start=(t == 0), stop=(t == 1),
                    )
            nc.vector.tensor_reduce(
                vph[:, ck : ck + 2, :].rearrange(
                    "p c (h2 w2) -> p c h2 w2", h2=4
                ),
                pv2[:, :, :].rearrange(
                    "p c (h2 dh w2 dw) -> p c h2 w2 dh dw",
                    h2=4, dh=2, w2=32, dw=2,
                ),
                axis=mybir.AxisListType.XY, op=Max,
            )
            for h in range(2):
                ptr = pWsm.tile([128, 128], bf16, tag="sm")
                nc.tensor.transpose(ptr[:], vph[:, ck + h, :], idb[:])
                nc.scalar.copy(vT[:, (ck + h) // 2, (ck + h) % 2, :], ptr[:])
                if ck + h >= KT - 2:
                    nc.scalar.copy(vTb[:, (ck + h) % 2, :], ptr[:])

        def submax_phase(s):
            qs, kph = qs_l[s], kph_l[s]
            ksub = kph[0:32, :, :].rearrange(
                "p kt (j v) -> p kt j v", v=16
            )[:, :, :, 0]
            cmax = cm.tile([128, 32], bf16, tag="cmax")
            for ck in range(0, NCHUNK, 4):
                psm = pBig.tile([128, 16, 64], f32, tag="big")
                for jj in range(16):
                    c2, j = ck + jj // 4, jj % 4
                    nc.tensor.matmul(
                        psm[:, jj, :],
                        qs[0:32, c2, j * 128 : (j + 1) * 128],
                        ksub,
                        start=True, stop=True,
                    )
                nc.vector.tensor_reduce(
                    cmax[:, ck * 4 : ck * 4 + 16],
                    psm[:],
                    axis=mybir.AxisListType.X, op=Max,
                )
            cneg = cm.tile([128, 32], bf16, tag="cneg")
            nc.vector.tensor_scalar(
                cneg[:], cmax[:], -1.0, -SHIFT_DELTA, Mult, Add
            )
            pcn = pWsm.tile([32, 128], bf16, tag="sm")
            nc.tensor.transpose(pcn[:], cneg[:], idb[:])
            cnT = cm.tile([32, 128], bf16, tag="cnT")
            nc.vector.tensor_copy(cnT[:], pcn[:])
            nc.sync.dma_start(
                qs[32:33, :, :].rearrange("o kt (j m) -> o (kt j) m", j=4),
                cnT[:],
            )

        # ---- attention phases: software pipeline, tails LAG chunks behind
        # their heads so the in-order PE never blocks on the Act exp stream.
        jobs = [(s, ck) for s in range(SPC) for ck in range(NCHUNK)]
        LAG = 4
        pend = {}

        def emit_head(i):
            s, ck = jobs[i]
            qs, kph = qs_l[s], kph_l[s]
            egs = []
            for g in range(NPAIR):
                pa = pBig.tile([128, 2, CHUNK], f32, tag="big")
                for t in range(2):
                    nc.tensor.matmul(
                        pa[:, t, :],
                        kph[:, 2 * g + t, :],
                        qs[:, ck, :],
                        start=True, stop=True,
                    )
                if g < NPAIR - 1 or (i % 2 == 0) or i >= 14:
                    eg = ep.tile([128, 2, CHUNK], e5, tag="E")
                    nc.scalar.activation(eg[:], pa[:], Exp)
                else:
                    # Schraudolph exp in bf16 bits on the DVE:
                    # bits16 = rne(z*184.665 + 16250.5); bitcast -> bf16
                    eg = ep.tile([128, 2, CHUNK], i16, tag="E")
                    nc.vector.tensor_scalar(
                        eg[:], pa[:], 184.6650, 16250.5, Mult, Add
                    )
                    eg = eg.bitcast(bf16)
                egs.append(eg)
            pend[i] = (egs, i % 2 != 0 and i < 14)

        rbs = {}

        def emit_chain(i):
            egs, last_bf = pend[i]
            den = pWsm.tile([128, 4], f32, tag="sm", name=f"den{i}")
            for j in range(4):
                for g in range(NPAIR - 1):
                    nc.tensor.matmul(
                        den[:, j : j + 1],
                        egs[g][:, :, j * 128 : (j + 1) * 128],
                        on8[:],
                        start=(g == 0), stop=False,
                        perf_mode=DR,
                    )
                if last_bf:
                    for t in range(2):
                        nc.tensor.matmul(
                            den[:, j : j + 1],
                            egs[NPAIR - 1][:, t, j * 128 : (j + 1) * 128],
                            onb[:],
                            start=False, stop=(t == 1),
                        )
                else:
                    nc.tensor.matmul(
                        den[:, j : j + 1],
                        egs[NPAIR - 1][:, :, j * 128 : (j + 1) * 128],
                        on8[:],
                        start=False, stop=True,
                        perf_mode=DR,
                    )
            r4 = rp.tile([128, 4], f32, tag="r4", name=f"r4_{i}")
            nc.vector.reciprocal_approx_fast(r4[:], den[:])
            prT = pWsm.tile([4, 128], f32, tag="sm", name=f"prT{i}")
            nc.tensor.transpose(prT[:], r4[:], idf[:])
            rr4 = rp.tile([4, 128], f32, tag="rr4", name=f"rr4_{i}")
            nc.vector.tensor_copy(rr4[:], prT[:])
            rrow = rp.tile([1, CHUNK], f32, tag="rrow", name=f"rrow{i}")
            nc.sync.dma_start(
                rrow[0:1, :].rearrange("o (j m) -> o j m", j=4), rr4[:]
            )
            rb = rp.tile([128, CHUNK], f32, tag="rb", name=f"rb{i}")
            nc.gpsimd.partition_broadcast(rb[:], rrow[0:1, :])
            rbs[i] = rb

        uns = {}

        def emit_U(i):
            s, ck = jobs[i]
            egs, last_bf = pend[i]
            vT = vT_l[s]
            rb = rbs.pop(i)
            pT = pBig if i >= len(jobs) - 4 else pWu
            pu = pT.tile([128, CHUNK], f32, name=f"pu{i}", tag="u" if pT is pWu else "big")
            for g in range(NPAIR - 1):
                nc.tensor.matmul(
                    pu[:], vT[:, g, :, :], egs[g][:],
                    start=(g == 0), stop=False,
                    perf_mode=DR,
                )
            if last_bf:
                vTb = vTb_l[s]
                for t in range(2):
                    nc.tensor.matmul(
                        pu[:], vTb[:, t, :], egs[NPAIR - 1][:, t, :],
                        start=False, stop=(t == 1),
                    )
            else:
                nc.tensor.matmul(
                    pu[:], vT[:, NPAIR - 1, :, :], egs[NPAIR - 1][:],
                    start=False, stop=True,
                    perf_mode=DR,
                )
            un = up.tile([128, CHUNK], e4, tag="un", name=f"un{i}")
            nc.vector.tensor_mul(un[:], pu[:], rb[:])
            uns[i] = un

        def emit_tail(i):
            s, ck = jobs[i]
            if i not in rbs and i not in uns:
                emit_chain(i)
            cs = slice(ck * CHUNK, (ck + 1) * CHUNK)
            if i not in uns:
                emit_U(i)
            un = uns.pop(i)
            pend.pop(i)
            if i == 11:
                # drain hoist: chains AND U/normalize of tails 12-15 emitted
                # here so they overlap the last heads' exps; the drain then
                # only runs Wa/residual/store per tail
                for j2 in range(12, len(jobs)):
                    emit_chain(j2)
                for j2 in range(12, len(jobs)):
                    emit_U(j2)
            pT = pBig if i >= len(jobs) - 4 else pWu
            yt = yp.tile([128, 2, CHUNK], bf16, tag="y")
            drain = i >= len(jobs) - 5
            for mt in range(2):
                po = pT.tile([128, CHUNK], f32, name=f"po{i}_{mt}", tag="u" if pT is pWu else "big")
                nc.tensor.matmul(
                    po[:], wa[:, mt, :], un[:],
                    start=True, stop=True,
                )
                if drain and mt == 1:
                    # drain is DVE-bound while Act idles: evacuate the second
                    # po on Act (in parallel with DVE's first residual add),
                    # then its residual add runs in 2x bf16 mode on DVE
                    tp = yp.tile(
                        [128, CHUNK], bf16, tag="ytmp", name=f"ytmp{i}_{mt}"
                    )
                    nc.scalar.copy(tp[:], po[:])
                    nc.vector.tensor_add(
                        yt[:, mt, :], tp[:], xrs[s][:, mt, cs]
                    )
                else:
                    nc.vector.tensor_add(
                        yt[:, mt, :], po[:], xrs[s][:, mt, cs]
                    )
            nc.sync.dma_start(
                y_d[s, :, :, cs].rearrange("t p m -> p t m"), yt[:]
            )

        load_x(0, ((0, 1024), (1024, 2048), (2048, 4096)))
        for ck in range(0, NCHUNK, 2):
            conv_chunk2(0, ck)
        submax_phase(0)
        load_x(1, ((0, 1024), (1024, 2048), (2048, 4096)))
        for ck in range(0, NCHUNK, 2):
            conv_chunk2(1, ck)
        submax_phase(1)
        for i in range(len(jobs) + LAG):
            if i < len(jobs):
                emit_head(i)
            if i >= LAG:
                emit_tail(i - LAG)

    nc.compile()
    return nc


def _get_program():
    if "nc" not in _built:
        _built["nc"] = _build_program()
    return _built["nc"]


def _make_in_maps(x, Wq, Wk, Wv, Wa, gamma):
    import ml_dtypes

    x = np.ascontiguousarray(
        np.asarray(x, dtype=np.float32)
        .astype(ml_dtypes.bfloat16)
        .reshape(B, 2, 128, HWF)
    )
    wqkT = np.concatenate([np.asarray(Wq), np.asarray(Wk)], axis=0).T
    wqkT = np.ascontiguousarray(
        wqkT.reshape(2, 128, 64).transpose(1, 0, 2).astype(ml_dtypes.bfloat16)
    )
    wvT = np.ascontiguousarray(
        np.asarray(Wv).T.reshape(2, 128, 128)
        .transpose(1, 0, 2).astype(ml_dtypes.bfloat16)
    )
    g = float(np.asarray(gamma).reshape(-1)[0])
    waT = np.ascontiguousarray(
        (g * np.asarray(Wa)).T.reshape(128, 2, 128).astype(ml_dtypes.bfloat16)
    )
    identB = np.eye(128, dtype=np.float32).astype(ml_dtypes.bfloat16)
    identF = np.eye(128, dtype=np.float32)
    blob = np.zeros((128, 2048), dtype=np.uint8)
    blob[:, 0:256] = wqkT.view(np.uint8).reshape(128, 256)
    blob[:, 256:768] = wvT.view(np.uint8).reshape(128, 512)
    blob[:, 768:1280] = waT.view(np.uint8).reshape(128, 512)
    blob[:, 1280:1536] = identB.view(np.uint8).reshape(128, 256)
    blob[:, 1536:2048] = identF.view(np.uint8).reshape(128, 512)
    return [
        {
            "x": np.ascontiguousarray(x[c * SPC : (c + 1) * SPC]),
            "cblob": blob,
        }
        for c in range(NCORES)
    ]


def kernel(x, Wq, Wk, Wv, Wa, gamma):
    from concourse import bass_utils

    nc = _get_program()
    in_maps = _make_in_maps(x, Wq, Wk, Wv, Wa, gamma)
    res = bass_utils.run_bass_kernel_spmd(
        nc, in_maps, core_ids=list(range(NCORES))
    )
    out = np.concatenate(
        [
            np.asarray(res.results[c]["y"]).astype(np.float32).reshape(1, SPC, C, HWF)
            for c in range(NCORES)
        ],
        axis=0,
    ).reshape(B, C, H, W)
    return out
